# revision 36
# baseline (speedup 1.0000x reference)
"""Trainium2 Bass kernel for nn_CrossModalFusionModel (sparse sliding-window
cross-attention, 2 modules: image<-text and text<-image).

Sharding: head-parallel tensor parallelism over 8 NeuronCores. Core h owns
attention head h (dh=128) of BOTH modules: it computes its head's Q/K/V with
host-folded projection weights (input-proj and attention-proj chains collapse
into one matmul), runs banded attention for that head, and emits a full-D
o-projection partial plus its D-slice of the residual projection. The host
sums the 8 partials (the unshard step). No collectives.

Perf structure (cost-model driven; PE sequencer is ~70ns/matmul, engine is
free-size * 0.42ns, DMA is ~360GB/s single device + 625ns/DMA desc-gen):
- fp16 wire + compute: 1 cycle/row matmuls at any free size, 2x/4x DVE.
- Banded attention: scores per KEY-tile c as ONE wide matmul over that
  tile's 2-3 consecutive query tiles; the band mask for a group is a
  contiguous slice of one [d+1|d0|d-1] pattern tile.
- ssum / AV accumulate with a rank-1 zero-matmul to open the PSUM region,
  then one wide matmul per key-group and ONE full-width pad matmul
  (pad weight ePm is zero for interior queries).
- V projected transposed (wide matmuls) then flipped via 4 PE transposes.
- The reference's zero-pad slots collapse into one virtual pad column
  per query with weight n_pad(i).
- All dram tensors host-packed partition-major; small constants in 2 DMAs.
- PE warmup matmuls seeded from an on-chip memset burn the cost model's
  pe-ramp window while the first DMAs land.
"""

import math

import numpy as np
import ml_dtypes

N = 512          # tokens / patches
DM = 1024        # d_model
DH = 128         # head dim
NT = N // 128    # 4 tiles
C_IMG = 1024
C_TXT = 768
WINDOW = 64
NCORES = 8

# per query-tile qt, the aligned 128-key tiles covering its 66-band
CHUNKS = [[0, 1], [0, 1, 2], [1, 2, 3], [2, 3]]
# per key-tile c, the consecutive query tiles it serves
CQTS = [[0, 1], [0, 1, 2], [1, 2, 3], [2, 3]]
GW = [len(q) * 128 for q in CQTS]            # group widths
GBASE = np.cumsum([0] + GW).tolist()
NG = GBASE[-1]                               # 1280

# compute dtype knob: "f16" | "bf16"
COMPUTE_DTYPE = "f16"
WARMUP_MM = 7
FP8_QKV = True      # q/k/v projections in fp8e4m3 + DoubleRow
FP8_SCALE = 16.0    # folded into wq/wk/wv (and unfolded via Exp scale / wo)
DUMMY_CHAIN = True   # warm rhs from a dummy Exp (early ACT table load)
INTERLEAVE = True    # interleave the two modules' attention chains
DMA_V2 = False       # attention inputs first, fp16 x + rw last
XI8_EARLY = True     # xi8 right after xt8

_prog_cache = {}
LAST_RESULT = {}

# cpack layout (CD [128, CP_LEN])
CP_MASK3 = 0
CP_IDENT = 384
CP_COLC = 512          # kbc_ia | kbc_ta | ones_col
CP_LEN = 515
# colf (f32 [128, 8]): bq_ia bk_ia bq_ta bk_ta brx brt bv_ia bv_ta
# rowpack layout (CD [1, RP_LEN])
RP_MASKP = 0
RP_VBP = {"ia": 512, "ta": 512 + DH}
RP_ONES = 512 + 2 * DH
RP_LEN = RP_ONES + DH


def _np_cd(cd):
    return {"f16": np.float16, "bf16": ml_dtypes.bfloat16}[cd]


def _host_cd(x, cd):
    return np.ascontiguousarray(np.ascontiguousarray(x).astype(_np_cd(cd)))


def _pack_pm(w, cd):
    """[C, X] -> partition-major [128, (C//128)*X] so DMA lines are long."""
    C, X = w.shape
    return _host_cd(w.reshape(C // 128, 128, X).transpose(1, 0, 2)
                    .reshape(128, (C // 128) * X), cd)


def _pack_pm8(w):
    C, X = w.shape
    out = (np.ascontiguousarray(w).reshape(C // 128, 128, X)
           .transpose(1, 0, 2).reshape(128, (C // 128) * X))
    return np.ascontiguousarray(out.astype(np.float32)
                                .astype(ml_dtypes.float8_e4m3))


def _q8(x):
    return x.astype(np.float32).astype(ml_dtypes.float8_e4m3)


def _hilo8(w):
    """w -> (hi, lo) fp8 pair with hi + lo ~= w."""
    hi = _q8(w)
    lo = _q8(w - hi.astype(np.float64))
    return hi, lo


def _build_program(cd):
    import concourse.bass as bass
    import concourse.tile as tile
    from concourse import bacc, mybir

    f32 = mybir.dt.float32
    CD = {"f16": mybir.dt.float16, "bf16": mybir.dt.bfloat16}[cd]
    Exp = mybir.ActivationFunctionType.Exp
    Copy = mybir.ActivationFunctionType.Copy
    Ident = mybir.ActivationFunctionType.Identity

    nc = bacc.Bacc("TRN2", target_bir_lowering=False, debug=False,
                   num_devices=NCORES)

    def din(name, shape, dt=CD):
        return nc.dram_tensor(name, shape, dt, kind="ExternalInput")

    def dout(name, shape, dt=CD):
        return nc.dram_tensor(name, shape, dt, kind="ExternalOutput")

    CD8 = mybir.dt.float8e4
    WD = CD8 if FP8_QKV else CD
    d_xt8 = din("xt8", [128, 6 * N], WD)
    d_xi8 = din("xi8", [128, 8 * N], WD)
    dw = {}
    for m, cq, cc in (("ia", C_IMG, C_TXT), ("ta", C_TXT, C_IMG)):
        dw[m] = dict(
            wq=din(f"wq_{m}", [128, (cq // 128) * DH], WD),
            wk=din(f"wk_{m}", [128, (cc // 128) * DH], WD),
            wv=din(f"wv_{m}", [128, (cc // 128) * DH], WD),
            wo=din(f"wo_{m}", [DH, DM]),
        )
    d_xl8_i = din("xl8_i", [128, 8 * N], WD)
    d_xl8_t = din("xl8_t", [128, 6 * N], WD)
    d_rwh_i = din("rwh_i", [128, 8 * DH], WD)
    d_rwl_i = din("rwl_i", [128, 8 * DH], WD)
    d_rwh_t = din("rwh_t", [128, 6 * DH], WD)
    d_rwl_t = din("rwl_t", [128, 6 * DH], WD)
    d_cpack = din("cpack", [128, CP_LEN])
    d_colf = din("colf", [128, 8], f32)
    d_rowp = din("rowp", [1, RP_LEN])

    d_po = {m: dout(f"po_{m}", [DM, N]) for m in ("ia", "ta")}
    d_xr = dout("xr", [128, 2, N])

    with tile.TileContext(nc) as tc:
        with tc.tile_pool(name="consts", bufs=1) as consts, \
             tc.tile_pool(name="work", bufs=3) as work, \
             tc.tile_pool(name="ps_big", bufs=5, space="PSUM") as ps_big, \
             tc.tile_pool(name="ps_rx", bufs=1, space="PSUM") as ps_rx, \
             tc.tile_pool(name="ps_row", bufs=2, space="PSUM") as ps_row:

            # ---- PE warmup seeded from an on-chip memset (no DMA dep).
            # The warm rhs comes from a dummy Exp so the ACT function-table
            # load (1283ns) is forced into the idle startup window.
            seed = consts.tile([128, N], CD, tag="seed")
            nc.gpsimd.memset(seed[:], 0.0)
            if DUMMY_CHAIN:
                dummy = consts.tile([1, N], CD, tag="dummy")
                nc.scalar.activation(dummy[:], seed[0:1, :], Exp)
            if WARMUP_MM:
                warm_ps = ps_big.tile([128, N], f32, tag="big")
                for _ in range(WARMUP_MM):
                    if DUMMY_CHAIN:
                        nc.tensor.matmul(warm_ps[:], seed[0:1, 0:128],
                                         dummy[:], start=True, stop=True)
                    else:
                        nc.tensor.matmul(warm_ps[:], seed[:, 0:128], seed[:],
                                         start=True, stop=True)

            def zrow(w):
                return seed[0:1, 0:w]

            def load_pm(name, dram, nct, x, dt=CD):
                t = consts.tile([128, nct, x], dt, tag=name, name=name)
                src = dram.ap().rearrange("p (c x) -> p c x", x=x)
                if x == N and nct >= 6:
                    h = nct // 2
                    nc.sync.dma_start(t[:, 0:h, :], src[:, 0:h, :])
                    nc.sync.dma_start(t[:, h:nct, :], src[:, h:nct, :])
                else:
                    nc.sync.dma_start(t[:], src)
                return t

            # ---- DMAs; small consts ride the idle Pool SWDGE queue ----
            cpack = consts.tile([128, CP_LEN], CD, tag="cpack")
            nc.gpsimd.dma_start(cpack[:], d_cpack[:])
            colf = consts.tile([128, 8], f32, tag="colf")
            nc.gpsimd.dma_start(colf[:], d_colf[:])
            rowp = consts.tile([1, RP_LEN], CD, tag="rowp")
            nc.gpsimd.dma_start(rowp[:], d_rowp[:])
            if DMA_V2:
                wq_ta = load_pm("wq_ta", dw["ta"]["wq"], 6, DH, WD)
                xt8 = load_pm("xt8", d_xt8, 6, N, WD)
                wk_ia = load_pm("wk_ia", dw["ia"]["wk"], 6, DH, WD)
                wv_ia = load_pm("wv_ia", dw["ia"]["wv"], 6, DH, WD)
                wq_ia = load_pm("wq_ia", dw["ia"]["wq"], 8, DH, WD)
                xi8 = load_pm("xi8", d_xi8, 8, N, WD)
                wk_ta = load_pm("wk_ta", dw["ta"]["wk"], 8, DH, WD)
                wv_ta = load_pm("wv_ta", dw["ta"]["wv"], 8, DH, WD)
                wo_ia = consts.tile([DH, DM], CD, tag="wo_ia")
                nc.sync.dma_start(wo_ia[:], dw["ia"]["wo"][:])
                wo_ta = consts.tile([DH, DM], CD, tag="wo_ta")
                nc.sync.dma_start(wo_ta[:], dw["ta"]["wo"][:])
                xl8_t = load_pm("xl8_t", d_xl8_t, 6, N, WD)
                xl8_i = load_pm("xl8_i", d_xl8_i, 8, N, WD)
                rwh_t = load_pm("rwh_t", d_rwh_t, 6, DH, WD)
                rwl_t = load_pm("rwl_t", d_rwl_t, 6, DH, WD)
                rwh_i = load_pm("rwh_i", d_rwh_i, 8, DH, WD)
                rwl_i = load_pm("rwl_i", d_rwl_i, 8, DH, WD)
            elif XI8_EARLY:
                wq_ta = load_pm("wq_ta", dw["ta"]["wq"], 6, DH, WD)
                xt8 = load_pm("xt8", d_xt8, 6, N, WD)
                wq_ia = load_pm("wq_ia", dw["ia"]["wq"], 8, DH, WD)
                xi8 = load_pm("xi8", d_xi8, 8, N, WD)
                wk_ia = load_pm("wk_ia", dw["ia"]["wk"], 6, DH, WD)
                wv_ia = load_pm("wv_ia", dw["ia"]["wv"], 6, DH, WD)
                wk_ta = load_pm("wk_ta", dw["ta"]["wk"], 8, DH, WD)
                wv_ta = load_pm("wv_ta", dw["ta"]["wv"], 8, DH, WD)
                wo_ia = consts.tile([DH, DM], CD, tag="wo_ia")
                nc.sync.dma_start(wo_ia[:], dw["ia"]["wo"][:])
                xl8_t = load_pm("xl8_t", d_xl8_t, 6, N, WD)
                wo_ta = consts.tile([DH, DM], CD, tag="wo_ta")
                nc.sync.dma_start(wo_ta[:], dw["ta"]["wo"][:])
                xl8_i = load_pm("xl8_i", d_xl8_i, 8, N, WD)
                rwh_t = load_pm("rwh_t", d_rwh_t, 6, DH, WD)
                rwl_t = load_pm("rwl_t", d_rwl_t, 6, DH, WD)
                rwh_i = load_pm("rwh_i", d_rwh_i, 8, DH, WD)
                rwl_i = load_pm("rwl_i", d_rwl_i, 8, DH, WD)
            else:
                wq_ta = load_pm("wq_ta", dw["ta"]["wq"], 6, DH, WD)
                xt8 = load_pm("xt8", d_xt8, 6, N, WD)
                wk_ia = load_pm("wk_ia", dw["ia"]["wk"], 6, DH, WD)
                wv_ia = load_pm("wv_ia", dw["ia"]["wv"], 6, DH, WD)
                wq_ia = load_pm("wq_ia", dw["ia"]["wq"], 8, DH, WD)
                xi8 = load_pm("xi8", d_xi8, 8, N, WD)
                wk_ta = load_pm("wk_ta", dw["ta"]["wk"], 8, DH, WD)
                wv_ta = load_pm("wv_ta", dw["ta"]["wv"], 8, DH, WD)
                wo_ia = consts.tile([DH, DM], CD, tag="wo_ia")
                nc.sync.dma_start(wo_ia[:], dw["ia"]["wo"][:])
                xl8_t = load_pm("xl8_t", d_xl8_t, 6, N, WD)
                wo_ta = consts.tile([DH, DM], CD, tag="wo_ta")
                nc.sync.dma_start(wo_ta[:], dw["ta"]["wo"][:])
                xl8_i = load_pm("xl8_i", d_xl8_i, 8, N, WD)
                rwh_t = load_pm("rwh_t", d_rwh_t, 6, DH, WD)
                rwl_t = load_pm("rwl_t", d_rwl_t, 6, DH, WD)
                rwh_i = load_pm("rwh_i", d_rwh_i, 8, DH, WD)
                rwl_i = load_pm("rwl_i", d_rwl_i, 8, DH, WD)

            mask3 = cpack[:, CP_MASK3:CP_MASK3 + 384]
            ident = cpack[:, CP_IDENT:CP_IDENT + 128]
            kbc = {"ia": cpack[:, CP_COLC:CP_COLC + 1],
                   "ta": cpack[:, CP_COLC + 1:CP_COLC + 2]}
            ones_col = cpack[:, CP_COLC + 2:CP_COLC + 3]
            bq = {"ia": colf[:, 0:1], "ta": colf[:, 2:3]}
            bk = {"ia": colf[:, 1:2], "ta": colf[:, 3:4]}
            brx = colf[:, 4:5]
            brt = colf[:, 5:6]
            bv = {"ia": colf[:, 6:7], "ta": colf[:, 7:8]}
            ones_row = rowp[:, RP_ONES:RP_ONES + DH]
            maskP = rowp[:, RP_MASKP:RP_MASKP + N]
            vbp = {m: rowp[:, RP_VBP[m]:RP_VBP[m] + DH] for m in ("ia", "ta")}
            wo = {"ia": wo_ia, "ta": wo_ta}

            DR = mybir.MatmulPerfMode.DoubleRow

            def projT(w3, x3, nct, bias_col, tag, use_act=False,
                      out=None, oslice=None, fp8=False):
                """out^T [128, N] = (x @ W^T)^T + bias via ct chunks.
                fp8: 256-deep DoubleRow chunks (w3/x3 are fp8, nct even)."""
                ps = ps_big.tile([128, N], f32, tag="big")
                if fp8:
                    for c in range(nct // 2):
                        nc.tensor.matmul(ps[:], w3[:, 2 * c:2 * c + 2, :],
                                         x3[:, 2 * c:2 * c + 2, :],
                                         perf_mode=DR, start=(c == 0),
                                         stop=(c == nct // 2 - 1))
                else:
                    for ct in range(nct):
                        nc.tensor.matmul(ps[:], w3[:, ct, :], x3[:, ct, :],
                                         start=(ct == 0), stop=(ct == nct - 1))
                if out is None:
                    out = work.tile([128, N], CD, tag="sb_" + tag, name=tag)
                    oview = out[:]
                else:
                    oview = out[:, oslice, :] if oslice is not None else out[:]
                if use_act:
                    nc.scalar.activation(oview, ps[:], Ident, bias=bias_col)
                else:
                    nc.vector.tensor_scalar_add(oview, ps[:], bias_col[:])
                return out

            st = {}

            def vproj_T(m, x3, nct):
                """vT [dh, keys] via wide matmuls; bias fused in the cast."""
                st[m] = {}
                vps = ps_big.tile([128, N], f32, tag="big")
                if FP8_QKV:
                    for c in range(nct // 2):
                        nc.tensor.matmul(vps[:], wvs[m][:, 2 * c:2 * c + 2, :],
                                         x3[:, 2 * c:2 * c + 2, :],
                                         perf_mode=DR, start=(c == 0),
                                         stop=(c == nct // 2 - 1))
                else:
                    for ct in range(nct):
                        nc.tensor.matmul(vps[:], wvs[m][:, ct, :],
                                         x3[:, ct, :], start=(ct == 0),
                                         stop=(ct == nct - 1))
                vT = work.tile([128, N], CD, tag=f"vT_{m}", bufs=1,
                               name=f"vT_{m}")
                nc.vector.tensor_scalar_add(vT[:], vps[:], bv[m][:])
                st[m]["vT"] = vT

            def vproj_flip(m):
                """4 PE transposes into one PSUM tile -> vN [key%128, c*DH+d]."""
                vT = st[m]["vT"]
                tps = ps_big.tile([128, N], CD, tag="big")
                for c in range(NT):
                    nc.tensor.transpose(tps[:, c * DH:(c + 1) * DH],
                                        vT[:, c * 128:(c + 1) * 128], ident)
                vN = work.tile([128, NT * DH], CD, tag=f"vN_{m}", bufs=1,
                               name=f"vN_{m}")
                nc.vector.tensor_copy(vN[:], tps[:])
                st[m]["vN"] = vN

            def attn_scores(m, qT, kT):
                """Banded scores by key-tile group -> exp -> mask."""
                eT = work.tile([128, NG], CD, tag=f"eT_{m}", bufs=1,
                               name=f"eT_{m}")
                eTm = work.tile([128, NG], CD, tag=f"eTm_{m}", bufs=1,
                                name=f"eTm_{m}")
                for c in range(NT):
                    qts = CQTS[c]
                    gps = ps_big.tile([128, GW[c]], f32, tag="big")
                    nc.tensor.matmul(
                        gps[:], kT[:, c * 128:(c + 1) * 128],
                        qT[:, qts[0] * 128:(qts[-1] + 1) * 128],
                        start=True, stop=True)
                    sl = slice(GBASE[c], GBASE[c + 1])
                    nc.scalar.activation(eT[:, sl], gps[:], Exp,
                                         scale=DESCALE)
                    moff = (1 - (c - qts[0])) * 128
                    nc.vector.tensor_mul(eTm[:, sl], eT[:, sl],
                                         mask3[:, moff:moff + GW[c]])
                sp = ps_row.tile([1, N], f32, tag="row")
                nc.tensor.matmul(sp[:], kbc[m], qT[:], start=True, stop=True)
                eP = work.tile([1, N], CD, tag=f"eP_{m}", bufs=1,
                               name=f"eP_{m}")
                nc.scalar.activation(eP[:], sp[:], Exp, scale=DESCALE)
                ePm = work.tile([1, N], CD, tag=f"ePm_{m}", bufs=1,
                                name=f"ePm_{m}")
                nc.vector.tensor_mul(ePm[:], eP[:], maskP)
                st[m].update(eTm=eTm, ePm=ePm)

            def attn_ssum(m):
                """denominators: zero-open, one wide matmul per key group,
                one full-width pad matmul; -> rinv [1, N]."""
                eTm, ePm = st[m]["eTm"], st[m]["ePm"]
                ssum = ps_row.tile([1, N], f32, tag="row")
                nc.tensor.matmul(ssum[:], ones_col[0:1, :], ePm[:],
                                 start=True, stop=False, skip_group_check=True)
                for c in range(NT):
                    qts = CQTS[c]
                    nc.tensor.matmul(
                        ssum[:, qts[0] * 128:(qts[-1] + 1) * 128],
                        ones_col, eTm[:, GBASE[c]:GBASE[c + 1]],
                        start=False, stop=(c == NT - 1),
                        skip_group_check=True)
                rinv = work.tile([1, N], CD, tag=f"rinv_{m}", bufs=1,
                                 name=f"rinv_{m}")
                with nc.allow_low_precision(
                        reason="1/denom feeds a 16-bit matmul; denom O(10-100)"):
                    nc.vector.reciprocal(rinv[:], ssum[:])
                st[m]["rinv"] = rinv

            def attn_rbc(m):
                rps = ps_big.tile([128, N], f32, tag="big")
                nc.tensor.matmul(rps[:], ones_row, st[m]["rinv"][:],
                                 start=True, stop=True)
                rbc = work.tile([128, N], CD, tag=f"rbc_{m}", bufs=1,
                                name=f"rbc_{m}")
                nc.scalar.activation(rbc[:], rps[:], Copy)
                st[m]["rbc"] = rbc

            def attn_av(m):
                """onorm [128, N] = (V^T E^T) * rbc, zero-open + wide groups."""
                eTm, ePm, vN = st[m]["eTm"], st[m]["ePm"], st[m]["vN"]
                oT = ps_big.tile([128, N], f32, tag="big")
                nc.tensor.matmul(oT[:], vbp[m][0:1, :], ePm[:],
                                 start=True, stop=False, skip_group_check=True)
                for c in range(NT):
                    qts = CQTS[c]
                    nc.tensor.matmul(
                        oT[:, qts[0] * 128:(qts[-1] + 1) * 128],
                        vN[:, c * DH:(c + 1) * DH],
                        eTm[:, GBASE[c]:GBASE[c + 1]],
                        start=False, stop=(c == NT - 1),
                        skip_group_check=True)
                onorm = work.tile([128, N], CD, tag=f"onorm_{m}", bufs=1,
                                  name=f"onorm_{m}")
                nc.vector.tensor_mul(onorm[:], oT[:], st[m]["rbc"][:])
                st[m]["onorm"] = onorm

            def oproj_stage(m, half, eng="va", act_q=False):
                """one 2-chunk stage of the po partial [DM, N]; 1 DMA.
                eng: copy engines for the 2 chunks, from {v,a,p}."""
                onorm = st[m]["onorm"]
                stage = work.tile([128, 2, N], CD, tag="po_stage",
                                  bufs=8, name="po_stage")
                for k in range(2):
                    dt_i = half * 2 + k
                    pps = ps_big.tile([128, N], f32, tag="big")
                    nc.tensor.matmul(pps[:],
                                     wo[m][:, dt_i * 128:(dt_i + 1) * 128],
                                     onorm[:], start=True, stop=True)
                    if eng[k] == "v":
                        nc.vector.tensor_copy(stage[:, k, :], pps[:])
                    elif eng[k] == "a":
                        nc.scalar.activation(stage[:, k, :], pps[:], Copy)
                    else:
                        nc.gpsimd.tensor_copy(stage[:, k, :], pps[:])
                q = nc.scalar if act_q else nc.sync
                q.dma_start(
                    d_po[m].ap().rearrange("(c p) n -> p c n", p=128)
                    [:, half * 2:(half + 1) * 2, :], stage[:])

            xr_sb = work.tile([128, 2, N], CD, tag="xr_sb", bufs=1,
                              name="xr_sb")
            wvs = {"ia": wv_ia, "ta": wv_ta}

            # ---- interleaved schedule (program order == engine order) ----
            DESCALE = (1.0 / (FP8_SCALE * FP8_SCALE)) if FP8_QKV else 1.0

            def resid8_mm(rx_ps, wh, wl, xh, xl, nct, r0, r1):
                """resid*16 = xh@wh + xh@wl + xl@wh, fp8 DoubleRow passes;
                flat pass index r in [0, 3*nct//2)."""
                passes = [(wh, xh), (wl, xh), (wh, xl)]
                nh = nct // 2
                for r in range(r0, r1):
                    w3, x3 = passes[r // nh]
                    c = r % nh
                    nc.tensor.matmul(rx_ps[:], w3[:, 2 * c:2 * c + 2, :],
                                     x3[:, 2 * c:2 * c + 2, :], perf_mode=DR,
                                     start=(r == 0), stop=(r == 3 * nh - 1),
                                     skip_group_check=True)

            qT_ta = projT(wq_ta, xt8, 6, bq["ta"], "qta", fp8=FP8_QKV)
            kT_ia = projT(wk_ia, xt8, 6, bk["ia"], "kia", use_act=True,
                          fp8=FP8_QKV)
            vproj_T("ia", xt8, 6)
            qT_ia = projT(wq_ia, xi8, 8, bq["ia"], "qia", use_act=True,
                          fp8=FP8_QKV)
            vproj_flip("ia")
            attn_scores("ia", qT_ia, kT_ia)
            kT_ta = projT(wk_ta, xi8, 8, bk["ta"], "kta", fp8=FP8_QKV)
            if INTERLEAVE:
                vproj_T("ta", xi8, 8)
                vproj_flip("ta")
                attn_scores("ta", qT_ta, kT_ta)
                attn_ssum("ia")
                attn_rbc("ia")
                attn_ssum("ta")
                attn_rbc("ta")
                attn_av("ia")
                attn_av("ta")
                oproj_stage("ia", 0, "va")
                oproj_stage("ia", 1, "av")
                oproj_stage("ia", 2, "va")
                oproj_stage("ia", 3, "av")
            else:
                attn_ssum("ia")
                vproj_T("ta", xi8, 8)
                attn_rbc("ia")
                attn_av("ia")
                vproj_flip("ta")
                attn_scores("ta", qT_ta, kT_ta)
                oproj_stage("ia", 0, "va")
                oproj_stage("ia", 1, "av")
                attn_ssum("ta")
                oproj_stage("ia", 2, "va")
                oproj_stage("ia", 3, "av")
                attn_rbc("ta")
                attn_av("ta")
            # finale: oproj_ta stages as early as possible (onorm latency
            # filled by resid_img's first chunks), then ALL residual matmuls
            # last so the po DMA chains drain under them; xr DMA is the tail
            rx_ps = ps_rx.tile([128, N], f32, tag="rx")
            resid8_mm(rx_ps, rwh_i, rwl_i, xi8, xl8_i, 8, 0, 4)
            oproj_stage("ta", 0, "va")
            oproj_stage("ta", 1, "av")
            oproj_stage("ta", 2, "va")
            oproj_stage("ta", 3, "av")
            resid8_mm(rx_ps, rwh_i, rwl_i, xi8, xl8_i, 8, 4, 12)
            nc.scalar.activation(xr_sb[:, 0, :], rx_ps[:], Ident, bias=brx,
                                 scale=1.0 / FP8_SCALE)
            rt_ps = ps_rx.tile([128, N], f32, tag="rx")
            resid8_mm(rt_ps, rwh_t, rwl_t, xt8, xl8_t, 6, 0, 9)
            nc.scalar.activation(xr_sb[:, 1, :], rt_ps[:], Ident, bias=brt,
                                 scale=1.0 / FP8_SCALE)
            nc.sync.dma_start(d_xr[:], xr_sb[:])

    nc.compile()
    return nc


def _band_masks():
    """[128, 3*128] = [d+1 | d0 | d-1]; pattern for key tile c vs query
    tile qt is delta = c - qt, stored so a key-group's consecutive query
    tiles read one contiguous slice."""
    jj = np.arange(128)[:, None]
    ii = np.arange(128)[None, :]
    out = np.zeros((128, 3 * 128), dtype=np.float64)
    for i, d in enumerate((1, 0, -1)):
        delta = 128 * d + jj - ii
        out[:, i * 128:(i + 1) * 128] = ((delta >= -WINDOW // 2)
                                         & (delta <= WINDOW // 2 + 1))
    return out


def _npad():
    i = np.arange(N)
    lo = np.maximum(0, i - WINDOW // 2)
    hi = np.minimum(N - 1, i + WINDOW // 2 + 1)
    length = hi - lo + 1
    return np.maximum(0, WINDOW - length).astype(np.float64)


def kernel(**inputs):
    from concourse.bass_utils import run_bass_kernel_spmd

    cd = COMPUTE_DTYPE
    if cd not in _prog_cache:
        _prog_cache[cd] = _build_program(cd)
    nc = _prog_cache[cd]

    f8 = lambda x: np.asarray(x, dtype=np.float64)
    images = f8(inputs["images"])[0]        # [N, 1024]
    caps = f8(inputs["capitions"])[0]       # [N, 768]
    ip_w, ip_b = f8(inputs["ip_w"]), f8(inputs["ip_b"])
    tp_w, tp_b = f8(inputs["tp_w"]), f8(inputs["tp_b"])

    sc = 1.0 / math.sqrt(DH)
    s8 = FP8_SCALE if FP8_QKV else 1.0
    pack_w = _pack_pm8 if FP8_QKV else (lambda w: _pack_pm(w, cd))
    band = _band_masks()
    xi8h, xi8l = _hilo8(_pack_pm(images.T, "f16").astype(np.float64)
                        if False else images.T.reshape(8, 128, N)
                        .transpose(1, 0, 2).reshape(128, 8 * N))
    xt8h, xt8l = _hilo8(caps.T.reshape(6, 128, N)
                        .transpose(1, 0, 2).reshape(128, 6 * N))

    in_maps = []
    for h in range(NCORES):
        sl = slice(h * DH, (h + 1) * DH)
        cpack = np.zeros((128, CP_LEN), dtype=np.float64)
        cpack[:, CP_MASK3:CP_MASK3 + 384] = band
        cpack[:, CP_IDENT:CP_IDENT + 128] = np.eye(128)
        cpack[:, CP_COLC + 2] = 1.0
        colf = np.zeros((128, 8), dtype=np.float64)
        colf[:, 4] = ip_b[sl]
        colf[:, 5] = tp_b[sl]
        rowp = np.zeros((1, RP_LEN), dtype=np.float64)
        rowp[0, RP_MASKP:RP_MASKP + N] = _npad()
        rowp[0, RP_ONES:RP_ONES + DH] = 1.0
        im = {"xi8": np.ascontiguousarray(xi8h),
              "xt8": np.ascontiguousarray(xt8h),
              "xl8_i": np.ascontiguousarray(xi8l),
              "xl8_t": np.ascontiguousarray(xt8l)}
        for mi, (m, pw, pb, cw, cb) in enumerate(
                (("ia", ip_w, ip_b, tp_w, tp_b),
                 ("ta", tp_w, tp_b, ip_w, ip_b))):
            qw, qb = f8(inputs[f"{m}_qw"]), f8(inputs[f"{m}_qb"])
            kw, kb = f8(inputs[f"{m}_kw"]), f8(inputs[f"{m}_kb"])
            vw, vb = f8(inputs[f"{m}_vw"]), f8(inputs[f"{m}_vb"])
            ow = f8(inputs[f"{m}_ow"])
            im[f"wq_{m}"] = pack_w(((qw[sl] @ pw) * sc * s8).T)
            im[f"wk_{m}"] = pack_w(((kw[sl] @ cw) * s8).T)
            im[f"wv_{m}"] = pack_w(((vw[sl] @ cw) * s8).T)
            im[f"wo_{m}"] = _host_cd(ow[:, sl].T / s8, cd)
            colf[:, 2 * mi] = (qw[sl] @ pb + qb[sl]) * sc * s8
            colf[:, 2 * mi + 1] = (kw[sl] @ cb + kb[sl]) * s8
            cpack[:, CP_COLC + mi] = kb[sl] * s8
            colf[:, 6 + mi] = (vw[sl] @ cb + vb[sl]) * s8
            rowp[0, RP_VBP[m]:RP_VBP[m] + DH] = vb[sl] * s8
        rwh_i, rwl_i = _hilo8(ip_w[sl].T.reshape(8, 128, DH)
                              .transpose(1, 0, 2).reshape(128, 8 * DH) * s8)
        rwh_t, rwl_t = _hilo8(tp_w[sl].T.reshape(6, 128, DH)
                              .transpose(1, 0, 2).reshape(128, 6 * DH) * s8)
        im["rwh_i"] = np.ascontiguousarray(rwh_i)
        im["rwl_i"] = np.ascontiguousarray(rwl_i)
        im["rwh_t"] = np.ascontiguousarray(rwh_t)
        im["rwl_t"] = np.ascontiguousarray(rwl_t)
        im["cpack"] = _host_cd(cpack, cd)
        im["colf"] = np.ascontiguousarray(colf, dtype=np.float32)
        im["rowp"] = _host_cd(rowp, cd)
        in_maps.append(im)

    res = run_bass_kernel_spmd(nc, in_maps, list(range(NCORES)))
    LAST_RESULT["res"] = res

    outs = []
    for mi, m in enumerate(("ia", "ta")):
        acc = np.zeros((DM, N), dtype=np.float64)
        for h in range(NCORES):
            r = res.results[h]
            acc += r[f"po_{m}"].astype(np.float64)
            acc[h * DH:(h + 1) * DH] += r["xr"][:, mi, :].astype(np.float64)
        acc += f8(inputs["ia_ob" if m == "ia" else "ta_ob"])[:, None]
        outs.append(np.ascontiguousarray(acc.T[None]).astype(np.float32))
    return outs[0], outs[1]
